# revision 2
# baseline (speedup 1.0000x reference)
"""Trainium2 Bass kernel for a dense transformer block (B=8, N=1024, C=1024,
H=16, D=64, HID=4096) with padding-masked attention.

Sharding: data-parallel over batch — one batch element per NeuronCore (8 cores).
Transposed layout ([C partitions, N free]); matmuls contract over partitions.

Precision plan (rel err ~1.2e-2 vs 2e-2 gate, validated in numpy sim):
 - QKV / AV / proj: fp8e4m3 DoubleRow (2 k-tiles per instr, ~2x f32r rate).
   Weights scaled x64 host-side; unscale folded into post-matmul ops.
 - QK^T: bf16 (same PE rate as f32r, half the SBUF/DVE traffic).
 - fc1: bf16 x bf16 (exact enough; fp8 here would cost ~1.2e-2 err).
 - fc2: weight-compensated fp8 DoubleRow: two instr chains with (W_hi, W_lo)
   pair tiles ((Whi0,Whi1)@(h0,h1) + (Wlo0,Wlo1)@(h0,h1)) -> weight quant
   error ~0.1%; h stays single fp8 (the one surviving error source).
 - Attention softmax: exp in fp8 out; denominator via ones-column in v so
   numerator and denominator share the same quantized probs.
"""

import os
import sys

for _p in ("/opt/trn_rl_repo",):
    if _p not in sys.path:
        sys.path.insert(0, _p)
os.environ.setdefault("MYCRO_LOCAL_CACHE", "1")

import ml_dtypes  # noqa: E402
import numpy as np  # noqa: E402

import concourse.bacc as bacc  # noqa: E402
import concourse.tile as tile  # noqa: E402
from concourse import mybir  # noqa: E402
from concourse.bass_utils import run_bass_kernel_spmd  # noqa: E402

f32 = mybir.dt.float32
f32r = mybir.dt.float32r
bf16 = mybir.dt.bfloat16
fp8 = mybir.dt.float8e4
AF = mybir.ActivationFunctionType
ALU = mybir.AluOpType
PM = mybir.MatmulPerfMode
E4 = ml_dtypes.float8_e4m3

B, N, C = 8, 1024, 1024
H, D = 16, 64
HID = 4 * C
CT = C // 128           # 8 c-tiles
CP = CT // 2            # 4 c-tile pairs
NT = N // 128           # 8 n/k-tiles
HT = HID // 128         # 32 hid-tiles
HP = HT // 2            # 16 hid-tile pairs
SCALE = D ** -0.5
EPS = 1e-5
MASK_NEG = -10000.0
WS = 64.0               # fp8 weight scale
YS = 16.0               # attention-output scale into fp8

NCORES = 8


def _layer_norm(nc, tc, srcf, dstf, dst_dt, onesP, epsc, gc, bc, tag):
    """dstf(ct) <- LN(src) per token (free dim); srcf(ct) returns the ct'th
    [128, N] f32r slice (C on partitions). Stats via ones-matmul partition
    reductions broadcast to all 128 partitions."""
    ln_cm = tc.tile_pool(name=f"ln_{tag}", bufs=1)
    lnps_cm = tc.tile_pool(name=f"lnps_{tag}", bufs=1, space="PSUM")
    work = ln_cm.__enter__()
    ps = lnps_cm.__enter__()

    ps_sum = ps.tile([128, N], f32, tag="lnsum", name=f"ps_sum_{tag}")
    ps_sq = ps.tile([128, N], f32, tag="lnsq", name=f"ps_sq_{tag}")
    for ct in range(CT):
        sq = work.tile([128, N], f32r, tag="lnsqt", bufs=2, name=f"sq{tag}{ct}")
        nc.scalar.activation(out=sq, in_=srcf(ct), func=AF.Square)
        for ch in range(2):
            cs = slice(ch * 512, (ch + 1) * 512)
            nc.tensor.matmul(ps_sum[:, cs], lhsT=onesP, rhs=srcf(ct)[:, cs],
                             start=(ct == 0), stop=(ct == CT - 1))
            nc.tensor.matmul(ps_sq[:, cs], lhsT=onesP, rhs=sq[:, cs],
                             start=(ct == 0), stop=(ct == CT - 1))
    meanB = work.tile([128, N], f32, tag="meanB", name=f"meanB_{tag}")
    nc.vector.tensor_scalar_mul(meanB, ps_sum, 1.0 / C)
    msq = work.tile([128, N], f32, tag="msq", name=f"msq_{tag}")
    nc.scalar.activation(out=msq, in_=ps_sum, func=AF.Square, scale=1.0 / C)
    varB = work.tile([128, N], f32, tag="varB", name=f"varB_{tag}")
    nc.vector.scalar_tensor_tensor(varB, in0=ps_sq, scalar=1.0 / C, in1=msq,
                                   op0=ALU.mult, op1=ALU.subtract)
    stdB = work.tile([128, N], f32, tag="msq", name=f"stdB_{tag}")
    nc.scalar.activation(out=stdB, in_=varB, func=AF.Sqrt, bias=epsc,
                         scale=1.0)
    rstdB = work.tile([128, N], f32, tag="rstdB", name=f"rstdB_{tag}")
    nc.vector.reciprocal(rstdB, stdB)
    for ct in range(CT):
        d = work.tile([128, N], f32, tag="lnd", bufs=2, name=f"lnd{tag}{ct}")
        nc.vector.tensor_sub(d, srcf(ct), meanB)
        t = work.tile([128, N], f32, tag="lnt", bufs=2, name=f"lnt{tag}{ct}")
        nc.vector.scalar_tensor_tensor(t, in0=d, scalar=gc[:, ct:ct + 1],
                                       in1=rstdB, op0=ALU.mult, op1=ALU.mult)
        nc.scalar.activation(out=dstf(ct), in_=t, func=AF.Identity,
                             bias=bc[:, ct:ct + 1], scale=1.0)

    lnps_cm.__exit__(None, None, None)
    ln_cm.__exit__(None, None, None)


def _ln_from_stats(nc, work, ps_sum, ps_sq, srcf, dstf, onesP, epsc, gc, bc,
                   tag):
    meanB = work.tile([128, N], f32, tag="meanB", name=f"meanB_{tag}")
    nc.vector.tensor_scalar_mul(meanB, ps_sum, 1.0 / C)
    msq = work.tile([128, N], f32, tag="msq", name=f"msq_{tag}")
    nc.scalar.activation(out=msq, in_=ps_sum, func=AF.Square, scale=1.0 / C)
    varB = work.tile([128, N], f32, tag="varB", name=f"varB_{tag}")
    nc.vector.scalar_tensor_tensor(varB, in0=ps_sq, scalar=1.0 / C, in1=msq,
                                   op0=ALU.mult, op1=ALU.subtract)
    stdB = work.tile([128, N], f32, tag="msq", name=f"stdB_{tag}")
    nc.scalar.activation(out=stdB, in_=varB, func=AF.Sqrt, bias=epsc,
                         scale=1.0)
    rstdB = work.tile([128, N], f32, tag="rstdB", name=f"rstdB_{tag}")
    nc.vector.reciprocal(rstdB, stdB)
    for ct in range(CT):
        d = work.tile([128, N], f32, tag="lnd", bufs=2, name=f"lnd{tag}{ct}")
        nc.vector.tensor_sub(d, srcf(ct), meanB)
        t = work.tile([128, N], f32, tag="lnt", bufs=2, name=f"lnt{tag}{ct}")
        nc.vector.scalar_tensor_tensor(t, in0=d, scalar=gc[:, ct:ct + 1],
                                       in1=rstdB, op0=ALU.mult, op1=ALU.mult)
        nc.scalar.activation(out=dstf(ct), in_=t, func=AF.Identity,
                             bias=bc[:, ct:ct + 1], scale=1.0)


def build_program(repeat=1):
    nc = bacc.Bacc("TRN2", target_bir_lowering=False, debug=False)

    xT = nc.dram_tensor("xT", [C, N], f32, kind="ExternalInput").ap()
    maskv = nc.dram_tensor("maskv", [N], f32, kind="ExternalInput").ap()
    g1 = nc.dram_tensor("g1", [C], f32, kind="ExternalInput").ap()
    b1 = nc.dram_tensor("b1", [C], f32, kind="ExternalInput").ap()
    g2 = nc.dram_tensor("g2", [C], f32, kind="ExternalInput").ap()
    b2 = nc.dram_tensor("b2", [C], f32, kind="ExternalInput").ap()
    bproj = nc.dram_tensor("bproj", [C], f32, kind="ExternalInput").ap()
    bb1 = nc.dram_tensor("bb1", [HID], f32, kind="ExternalInput").ap()
    bb2 = nc.dram_tensor("bb2", [C], f32, kind="ExternalInput").ap()
    # fp8 weights, SBUF-layout ([128 partitions, ...]) packed host-side
    wqk8 = nc.dram_tensor("wqk8", [128, 16, CP, 256], fp8,
                          kind="ExternalInput").ap()
    wv8 = nc.dram_tensor("wv8", [128, CP, 2, 1024], fp8,
                         kind="ExternalInput").ap()
    wp8 = nc.dram_tensor("wp8", [128, CT, CP, 256], fp8,
                         kind="ExternalInput").ap()
    w2hi = nc.dram_tensor("w2hi", [128, CT, HP, 256], fp8,
                          kind="ExternalInput").ap()
    w2lo = nc.dram_tensor("w2lo", [128, CT, HP, 256], fp8,
                          kind="ExternalInput").ap()
    w1b = nc.dram_tensor("w1b", [8, CT, 128, 512], bf16,
                         kind="ExternalInput").ap()
    onesd = nc.dram_tensor("onesd", [1, 128], f32, kind="ExternalInput").ap()
    outT = nc.dram_tensor("outT", [C, N], f32, kind="ExternalOutput").ap()
    rrd = nc.dram_tensor("rrd", [H, N], f32)  # denom-reciprocal bounce

    # [C, N] DRAM tensors viewed as two [128, 4, N] row-groups for merged DMA
    def rg(ap_, half):
        return ap_[half * 512:(half + 1) * 512, :].rearrange(
            "(a p) f -> p a f", p=128)

    with tile.TileContext(nc) as tc:
        const_cm = tc.tile_pool(name="const", bufs=1)
        const = const_cm.__enter__()

        def vec_tiles(src_ap, n_t, name):
            t = const.tile([128, n_t], f32, name=name)
            nc.sync.dma_start(out=t, in_=src_ap.rearrange("(t p) -> p t", p=128))
            return t

        g1c = vec_tiles(g1, CT, "g1c")
        b1c = vec_tiles(b1, CT, "b1c")
        g2c = vec_tiles(g2, CT, "g2c")
        b2c = vec_tiles(b2, CT, "b2c")
        bprojc = vec_tiles(bproj, CT, "bprojc")
        bb1c = vec_tiles(bb1, HT, "bb1c")
        bb2c = vec_tiles(bb2, CT, "bb2c")
        maskc = vec_tiles(maskv, NT, "maskc")
        onesP = const.tile([128, 128], f32r, name="onesP")
        nc.sync.dma_start(out=onesP,
                          in_=onesd.partition_broadcast(128).bitcast(f32r))
        epsc = const.tile([128, 1], f32, name="epsc")
        nc.vector.memset(epsc, EPS)

        hoist = os.environ.get("KHOIST") == "1" and repeat > 1
        if hoist:
            hw_cm = tc.tile_pool(name="hoist_w", bufs=1)
            hwp = hw_cm.__enter__()
            h_wqk = hwp.tile([128, 16, CP, 256], fp8, name="h_wqk")
            nc.sync.dma_start(out=h_wqk[:, 0:8], in_=wqk8[:, 0:8])
            nc.sync.dma_start(out=h_wqk[:, 8:16], in_=wqk8[:, 8:16])
            h_wv = hwp.tile([128, CP, 2, 1024], fp8, name="h_wv")
            nc.gpsimd.dma_start(out=h_wv, in_=wv8)
            h_wp = hwp.tile([128, CT, CP, 256], fp8, name="h_wp")
            nc.sync.dma_start(out=h_wp, in_=wp8)
            h_w2hi = hwp.tile([128, CT, HP, 256], fp8, name="h_w2hi")
            h_w2lo = hwp.tile([128, CT, HP, 256], fp8, name="h_w2lo")
            for hh in range(2):
                nc.scalar.dma_start(out=h_w2hi[:, 4 * hh:4 * hh + 4],
                                    in_=w2hi[:, 4 * hh:4 * hh + 4])
                nc.sync.dma_start(out=h_w2lo[:, 4 * hh:4 * hh + 4],
                                  in_=w2lo[:, 4 * hh:4 * hh + 4])
            h_w1g = []
            for fg in range(8):
                tls = []
                for cg in range(2):
                    wt = hwp.tile([128, 4, 512], bf16, name=f"hw1_{fg}_{cg}")
                    nc.gpsimd.dma_start(
                        out=wt,
                        in_=w1b[fg, 4 * cg:4 * cg + 4].rearrange(
                            "a p f -> p a f"))
                    tls.append(wt)
                h_w1g.append(tls)

        for _rep in range(repeat):
            # ============ LN1 (x -> xn fp8 pairs) ====================
            # right-side pools, pushed in reverse order of lifetime
            px2t_cm = tc.tile_pool(name="p_x2t", bufs=1, side="right")
            px2t = px2t_cm.__enter__()
            x2tB = [px2t.tile([128, 4, N], f32r, tag="x2t", bufs=2,
                              name=f"x2t{g}") for g in range(2)]

            def x2slice(ct):
                return x2tB[ct // 4][:, ct % 4, :]

            pxts_cm = tc.tile_pool(name="p_xts", bufs=1, side="right")
            pxts = pxts_cm.__enter__()
            xtsB = []
            for g in range(2):
                t = pxts.tile([128, 4, N], f32r, tag="xts", bufs=2,
                              name=f"xts{g}")
                nc.scalar.dma_start(out=t, in_=rg(xT, g).bitcast(f32r))
                xtsB.append(t)

            def xslice(ct):
                return xtsB[ct // 4][:, ct % 4, :]

            pln1_cm = tc.tile_pool(name="p_ln1", bufs=1, side="left")
            pln1 = pln1_cm.__enter__()
            xn2 = [pln1.tile([128, 2, N], fp8, tag="xn2", bufs=CP,
                             name=f"xn2_{cp}") for cp in range(CP)]

            _layer_norm(nc, tc, xslice,
                        lambda ct: xn2[ct // 2][:, ct % 2, :], fp8,
                        onesP, epsc, g1c, b1c, "ln1")

            # ======================== QKV ============================
            pyt_cm = tc.tile_pool(name="p_yt", bufs=1, side="right")
            pyt = pyt_cm.__enter__()
            yt2 = [pyt.tile([128, 2, N], fp8, tag="yt2", bufs=CP,
                            name=f"yt2_{cp}") for cp in range(CP)]
            if hoist:
                wp_t = h_wp
            else:
                wp_t = pyt.tile([128, CT, CP, 256], fp8, name="wp_t")
                nc.sync.dma_start(out=wp_t, in_=wp8)

            pqk_cm = tc.tile_pool(name="p_qk", bufs=1, side="right")
            pqk = pqk_cm.__enter__()
            qkt = [pqk.tile([128, N], bf16, tag="qkt", bufs=16,
                            name=f"qkt{i}") for i in range(16)]
            vk2 = [pqk.tile([128, 2, H * (D + 1)], fp8, tag="vk2", bufs=4,
                            name=f"vk2_{kp}") for kp in range(CP)]
            for kp in range(CP):
                vcol = vk2[kp].rearrange("p t (h u) -> p t h u", u=D + 1)
                nc.vector.memset(vcol[:, :, :, D:D + 1], 1.0)

            wq_cm = tc.tile_pool(name="wq_pool", bufs=1)
            wqp = wq_cm.__enter__()
            if hoist:
                wqk_t, wv_t = h_wqk, h_wv
            else:
                wqk_t = wqp.tile([128, 16, CP, 256], fp8, name="wqk_t")
                nc.sync.dma_start(out=wqk_t[:, 0:8], in_=wqk8[:, 0:8])
                nc.sync.dma_start(out=wqk_t[:, 8:16], in_=wqk8[:, 8:16])
                wv_t = wqp.tile([128, CP, 2, 1024], fp8, name="wv_t")
                nc.gpsimd.dma_start(out=wv_t, in_=wv8)

            qps_cm = tc.tile_pool(name="qkv_ps", bufs=1, space="PSUM")
            qps = qps_cm.__enter__()

            def wqk_l(ot, cp):
                return wqk_t[:, ot, cp, :].rearrange("p (t f) -> p t f", t=2)

            def qk_one(ot):
                ps = qps.tile([128, N], f32, tag="qkvps", bufs=3,
                              name=f"qkps{ot}")
                for cp in range(CP):
                    for ch in range(2):
                        cs = slice(ch * 512, (ch + 1) * 512)
                        nc.tensor.matmul(ps[:, cs], lhsT=wqk_l(ot, cp),
                                         rhs=xn2[cp][:, :, cs],
                                         start=(cp == 0), stop=(cp == CP - 1),
                                         perf_mode=PM.DoubleRow)
                nc.vector.tensor_copy(qkt[ot], ps)

            def v_one(nt):
                ps = qps.tile([128, N], f32, tag="qkvps", bufs=3,
                              name=f"vps{nt}")
                ns = slice(nt * 128, (nt + 1) * 128)
                for cp in range(CP):
                    for ch in range(2):
                        cs = slice(ch * 512, (ch + 1) * 512)
                        nc.tensor.matmul(
                            ps[:, cs], lhsT=xn2[cp][:, :, ns],
                            rhs=wv_t[:, cp, ch, :].rearrange(
                                "p (t f) -> p t f", t=2),
                            start=(cp == 0), stop=(cp == CP - 1),
                            perf_mode=PM.DoubleRow)
                dst = vk2[nt // 2].rearrange(
                    "p t (h u) -> p t h u", u=D + 1)[:, nt % 2, :, 0:D]
                nc.scalar.activation(
                    out=dst, in_=ps.rearrange("p (h d) -> p h d", d=D),
                    func=AF.Identity, scale=1.0 / WS)

            # q0,k0 first so attention can start; v next (AV h0 needs all v)
            qk_one(0)
            qk_one(8)
            for nt in range(NT):
                v_one(nt)
            for j in range(1, 8):
                qk_one(j)
                qk_one(8 + j)

            qps_cm.__exit__(None, None, None)
            wq_cm.__exit__(None, None, None)
            pln1_cm.__exit__(None, None, None)

            # ====================== attention ========================
            asb_cm = tc.tile_pool(name="attn_sb", bufs=1)
            asb = asb_cm.__enter__()
            aps_cm = tc.tile_pool(name="attn_ps", bufs=1, space="PSUM")
            aps = aps_cm.__enter__()

            for h in range(H):
                j, half = h // 2, h % 2
                hs = slice(half * D, (half + 1) * D)
                qk_q = qkt[j]
                qk_k = qkt[8 + j]
                # all QK^T matmuls first; exp on ACT overlaps them
                ea2 = []
                for kt in range(NT):
                    ks = slice(kt * 128, (kt + 1) * 128)
                    sa = aps.tile([128, N], f32, tag="sa", bufs=3,
                                  name=f"sa{h}_{kt}")
                    for ch in range(2):
                        cs = slice(ch * 512, (ch + 1) * 512)
                        nc.tensor.matmul(sa[:, cs], lhsT=qk_k[hs, ks],
                                         rhs=qk_q[hs, cs],
                                         start=True, stop=True)
                    if kt % 2 == 0:
                        ea = asb.tile([128, 2, N], fp8, tag="ea", bufs=6,
                                      name=f"ea{h}_{kt // 2}")
                        ea2.append(ea)
                    nc.scalar.activation(out=ea2[kt // 2][:, kt % 2, :],
                                         in_=sa, func=AF.Exp,
                                         bias=maskc[:, kt:kt + 1],
                                         scale=SCALE / (WS * WS))
                yach = [aps.tile([D + 1, 512], f32, tag=f"ya{ch}", bufs=1,
                                 name=f"ya{h}_{ch}") for ch in range(2)]
                for kp in range(CP):
                    va = vk2[kp][:, :, h * 65:h * 65 + 65]
                    for ch in range(2):
                        cs = slice(ch * 512, (ch + 1) * 512)
                        nc.tensor.matmul(yach[ch], lhsT=va,
                                         rhs=ea2[kp][:, :, cs],
                                         start=(kp == 0), stop=(kp == CP - 1),
                                         perf_mode=PM.DoubleRow)
                yu = asb.tile([D + 1, N], f32, tag="yu", bufs=6,
                              name=f"yu{h}")
                for ch in range(2):
                    nc.vector.tensor_copy(yu[:, ch * 512:(ch + 1) * 512],
                                          yach[ch])
                rr = asb.tile([1, N], f32, tag="rr", bufs=6, name=f"rr{h}")
                nc.vector.reciprocal(rr, yu[D:D + 1, :])
                row = rrd.ap()[h:h + 1, :]
                nc.sync.dma_start(out=row, in_=rr)
                rb = asb.tile([D, N], f32, tag="rb", bufs=6, name=f"rb{h}")
                nc.sync.dma_start(out=rb, in_=row.partition_broadcast(D))
                # yt = y*YS/denom, fp8
                nc.vector.scalar_tensor_tensor(
                    yt2[h // 4][(h % 2) * D:(h % 2) * D + D, (h // 2) % 2, :],
                    in0=yu[0:D, :], scalar=YS, in1=rb,
                    op0=ALU.mult, op1=ALU.mult)

            aps_cm.__exit__(None, None, None)
            asb_cm.__exit__(None, None, None)
            pqk_cm.__exit__(None, None, None)

            # ============ proj + residual; prefetch MLP weights ======
            mw_cm = tc.tile_pool(name="mw_pool", bufs=1)
            mwp = mw_cm.__enter__()
            if hoist:
                w1groups = h_w1g
                w2hi_t, w2lo_t = h_w2hi, h_w2lo
            else:
                # w1 on gpsimd; w2hi on scalar, w2lo on sync
                w1groups = []
                for fg in range(8):
                    w1tiles = []
                    for cg in range(2):
                        wt = mwp.tile([128, 4, 512], bf16, tag="w1",
                                      bufs=4, name=f"w1_{fg}_{cg}")
                        nc.gpsimd.dma_start(
                            out=wt,
                            in_=w1b[fg, 4 * cg:4 * cg + 4].rearrange(
                                "a p f -> p a f"))
                        w1tiles.append(wt)
                    w1groups.append(w1tiles)
                w2hi_t = mwp.tile([128, CT, HP, 256], fp8, name="w2hi_t")
                w2lo_t = mwp.tile([128, CT, HP, 256], fp8, name="w2lo_t")
                for hh in range(2):
                    nc.scalar.dma_start(out=w2hi_t[:, 4 * hh:4 * hh + 4],
                                        in_=w2hi[:, 4 * hh:4 * hh + 4])
                    nc.sync.dma_start(out=w2lo_t[:, 4 * hh:4 * hh + 4],
                                      in_=w2lo[:, 4 * hh:4 * hh + 4])

            pps_cm = tc.tile_pool(name="proj_ps", bufs=1, space="PSUM")
            pps = pps_cm.__enter__()
            # LN2 stats psums + sq tiles interleaved with the proj loop so
            # the reduction starts as each x2 tile lands
            ln2w_cm = tc.tile_pool(name="ln2_w", bufs=1)
            ln2w = ln2w_cm.__enter__()
            ps2_sum = pps.tile([128, N], f32, tag="ln2sum", name="ps2_sum")
            ps2_sq = pps.tile([128, N], f32, tag="ln2sq", name="ps2_sq")
            for o in range(CT):
                ps = pps.tile([128, N], f32, tag="projps", bufs=2,
                              name=f"pps{o}")
                for cp in range(CP):
                    lw = wp_t[:, o, cp, :].rearrange("p (t f) -> p t f", t=2)
                    for ch in range(2):
                        cs = slice(ch * 512, (ch + 1) * 512)
                        nc.tensor.matmul(
                            ps[:, cs], lhsT=lw, rhs=yt2[cp][:, :, cs],
                            start=(cp == 0), stop=(cp == CP - 1),
                            perf_mode=PM.DoubleRow)
                # x2 = ps/(WS*YS) + x, then += bproj
                xs = x2slice(o)
                nc.vector.scalar_tensor_tensor(
                    xs, in0=ps, scalar=1.0 / (WS * YS),
                    in1=xslice(o), op0=ALU.mult, op1=ALU.add)
                nc.vector.tensor_scalar_add(xs, xs, bprojc[:, o:o + 1])
                sq = ln2w.tile([128, N], f32r, tag="ln2sqt", bufs=2,
                               name=f"sq2_{o}")
                nc.scalar.activation(out=sq, in_=xs, func=AF.Square)
                for ch in range(2):
                    cs = slice(ch * 512, (ch + 1) * 512)
                    nc.tensor.matmul(ps2_sum[:, cs], lhsT=onesP,
                                     rhs=xs[:, cs],
                                     start=(o == 0), stop=(o == CT - 1))
                    nc.tensor.matmul(ps2_sq[:, cs], lhsT=onesP,
                                     rhs=sq[:, cs],
                                     start=(o == 0), stop=(o == CT - 1))
            pyt_cm.__exit__(None, None, None)

            # =================== LN2 (x2 -> x2n bf16) ================
            px2n_cm = tc.tile_pool(name="p_x2n", bufs=1, side="left")
            px2n = px2n_cm.__enter__()
            x2nb = [px2n.tile([128, N], bf16, tag="x2n", bufs=CT,
                              name=f"x2n{ct}") for ct in range(CT)]
            _ln_from_stats(nc, ln2w, ps2_sum, ps2_sq, x2slice,
                           lambda ct: x2nb[ct], onesP, epsc, g2c, b2c, "ln2")
            pps_cm.__exit__(None, None, None)
            pxts_cm.__exit__(None, None, None)

            # ======== MLP: fc1 bf16, fc2 fp8 weight-compensated ======
            pmlp_cm = tc.tile_pool(name="p_mlp", bufs=1, side="right")
            pmlp = pmlp_cm.__enter__()
            h2 = [pmlp.tile([128, 2, N], fp8, tag="h2", bufs=HP,
                            name=f"h2_{fp}") for fp in range(HP)]
            mps_cm = tc.tile_pool(name="mlp_ps", bufs=1, space="PSUM")
            mps = mps_cm.__enter__()

            for f in range(HT):
                fg, fi = f // 4, f % 4
                w1tiles = w1groups[fg]
                ps = mps.tile([128, N], f32, tag="mlp1ps", bufs=2,
                              name=f"m1ps{f}")
                fs = slice(fi * 128, (fi + 1) * 128)
                for ct in range(CT):
                    for ch in range(2):
                        cs = slice(ch * 512, (ch + 1) * 512)
                        nc.tensor.matmul(
                            ps[:, cs], lhsT=w1tiles[ct // 4][:, ct % 4, fs],
                            rhs=x2nb[ct][:, cs],
                            start=(ct == 0), stop=(ct == CT - 1))
                nc.scalar.activation(out=h2[f // 2][:, f % 2, :], in_=ps,
                                     func=AF.Gelu, bias=bb1c[:, f:f + 1],
                                     scale=1.0)
            px2n_cm.__exit__(None, None, None)
            ln2w_cm.__exit__(None, None, None)

            # fc2 + bias + residual out
            osb_cm = tc.tile_pool(name="out_sb", bufs=1)
            osb = osb_cm.__enter__()
            ot = [osb.tile([128, 4, N], f32, tag="ot", bufs=2, name=f"ot{g}")
                  for g in range(2)]

            def w2_l(wt, o, hp):
                return wt[:, o, hp, :].rearrange("p (t f) -> p t f", t=2)

            for o in range(CT):
                pm = mps.tile([128, N], f32, tag="pm", bufs=2, name=f"pm{o}")
                for hp in range(HP):
                    for wt in (w2hi_t, w2lo_t):
                        lw = w2_l(wt, o, hp)
                        for ch in range(2):
                            cs = slice(ch * 512, (ch + 1) * 512)
                            nc.tensor.matmul(
                                pm[:, cs], lhsT=lw, rhs=h2[hp][:, :, cs],
                                start=(hp == 0 and wt is w2hi_t),
                                stop=(hp == HP - 1 and wt is w2lo_t),
                                perf_mode=PM.DoubleRow)
                dst = ot[o // 4][:, o % 4, :]
                nc.vector.scalar_tensor_tensor(
                    dst, in0=pm, scalar=1.0 / WS,
                    in1=x2slice(o), op0=ALU.mult, op1=ALU.add)
                nc.vector.tensor_scalar_add(dst, dst, bb2c[:, o:o + 1])
            for g in range(2):
                nc.scalar.dma_start(out=rg(outT, g), in_=ot[g])

            mps_cm.__exit__(None, None, None)
            osb_cm.__exit__(None, None, None)
            mw_cm.__exit__(None, None, None)
            pmlp_cm.__exit__(None, None, None)
            px2t_cm.__exit__(None, None, None)

        if hoist:
            hw_cm.__exit__(None, None, None)
        const_cm.__exit__(None, None, None)

    nc.compile()
    return nc


_NC_CACHE = {}


def _get_program():
    if "nc" not in _NC_CACHE:
        _NC_CACHE["nc"] = build_program()
    return _NC_CACHE["nc"]


def _prep_weights(Wqkv, Wproj, W1, W2):
    Wqkv = np.asarray(Wqkv, np.float32)
    Wproj = np.asarray(Wproj, np.float32)
    W1 = np.asarray(W1, np.float32)
    W2 = np.asarray(W2, np.float32)

    # q/k stationary: [p, ot, cp, t*f]; val = Wqk[ot*128+f, cp*256+t*128+p]*WS
    a = (Wqkv[:2 * C] * WS).reshape(16, 128, CP, 2, 128)   # [ot, f, cp, t, p]
    wqk8 = np.ascontiguousarray(
        a.transpose(4, 0, 2, 3, 1)).astype(E4).reshape(128, 16, CP, 256)
    # v moving: [p, cp, ch, t*n]; val = Wv[ch*512+n, cp*256+t*128+p]*WS
    a = (Wqkv[2 * C:] * WS).reshape(2, 512, CP, 2, 128)    # [ch, n, cp, t, p]
    wv8 = np.ascontiguousarray(
        a.transpose(4, 2, 0, 3, 1)).astype(E4).reshape(128, CP, 2, 1024)
    # proj stationary: [p, o, cp, t*f]
    a = (Wproj * WS).reshape(CT, 128, CP, 2, 128)          # [o, f, cp, t, p]
    wp8 = np.ascontiguousarray(
        a.transpose(4, 0, 2, 3, 1)).astype(E4).reshape(128, CT, CP, 256)
    # fc1 bf16, baseline-style [8, CT, 128, 512]
    W1T = np.ascontiguousarray(W1.T)                       # [C, HID]
    w1b = np.ascontiguousarray(
        W1T.reshape(CT, 128, 8, 512).transpose(2, 0, 1, 3)).astype(
        ml_dtypes.bfloat16)
    # fc2 stationary hi/lo: [p, o, hp, t*f]; val = W2[o*128+f, hp*256+t*128+p]
    a = (W2 * WS).reshape(CT, 128, HP, 2, 128)             # [o, f, hp, t, p]
    a = np.ascontiguousarray(a.transpose(4, 0, 2, 3, 1))   # [p, o, hp, t, f]
    hi = a.astype(E4)
    lo = (a - hi.astype(np.float32)).astype(E4)
    return {
        "wqk8": wqk8, "wv8": wv8, "wp8": wp8, "w1b": w1b,
        "w2hi": hi.reshape(128, CT, HP, 256),
        "w2lo": lo.reshape(128, CT, HP, 256),
    }


def kernel(x, length, g1, b1, Wqkv, Wproj, bproj, g2, b2, W1, bb1, W2, bb2):
    x = np.asarray(x, dtype=np.float32)
    length = np.asarray(length)
    g1 = np.asarray(g1, np.float32); b1 = np.asarray(b1, np.float32)
    g2 = np.asarray(g2, np.float32); b2 = np.asarray(b2, np.float32)
    bproj = np.asarray(bproj, np.float32)
    bb1 = np.asarray(bb1, np.float32); bb2 = np.asarray(bb2, np.float32)

    wts = _prep_weights(Wqkv, Wproj, W1, W2)
    xT = np.ascontiguousarray(x.transpose(0, 2, 1))  # [B, C, N]
    mask = (np.arange(N)[None, :] >= np.asarray(length)[:, None]).astype(
        np.float32) * MASK_NEG  # [B, N]

    shared = {"g1": g1, "b1": b1, "g2": g2, "b2": b2, "bproj": bproj,
              "bb1": bb1, "bb2": bb2, "onesd": np.ones((1, 128), np.float32),
              **wts}
    in_maps = [dict(shared, xT=xT[b], maskv=np.ascontiguousarray(mask[b]))
               for b in range(B)]

    nc = _get_program()
    res = run_bass_kernel_spmd(nc, in_maps, core_ids=list(range(NCORES)))
    out = np.stack([res.results[b]["outT"] for b in range(B)], axis=0)
    return np.ascontiguousarray(out.transpose(0, 2, 1))


# revision 3
# speedup vs baseline: 1.2443x; 1.2443x over previous
"""Trainium2 Bass kernel for a dense transformer block (B=8, N=1024, C=1024,
H=16, D=64, HID=4096) with padding-masked attention.

Sharding: data-parallel over batch — one batch element per NeuronCore (8 cores).
Transposed layout ([C partitions, N free]); matmuls contract over partitions.

Precision plan (rel err ~1.2e-2 vs 2e-2 gate, validated in numpy sim):
 - QKV / AV / proj: fp8e4m3 DoubleRow (2 k-tiles per instr, ~2x f32r rate).
   Weights scaled x64 host-side; unscale folded into post-matmul ops.
 - QK^T: bf16 (same PE rate as f32r, half the SBUF/DVE traffic).
 - fc1: bf16 x bf16 (exact enough; fp8 here would cost ~1.2e-2 err).
 - fc2: weight-compensated fp8 DoubleRow: two instr chains with (W_hi, W_lo)
   pair tiles ((Whi0,Whi1)@(h0,h1) + (Wlo0,Wlo1)@(h0,h1)) -> weight quant
   error ~0.1%; h stays single fp8 (the one surviving error source).
 - Attention softmax: exp in fp8 out; denominator via ones-column in v so
   numerator and denominator share the same quantized probs.
"""

import os
import sys

for _p in ("/opt/trn_rl_repo",):
    if _p not in sys.path:
        sys.path.insert(0, _p)
os.environ.setdefault("MYCRO_LOCAL_CACHE", "1")

import ml_dtypes  # noqa: E402
import numpy as np  # noqa: E402

import concourse.bacc as bacc  # noqa: E402
import concourse.tile as tile  # noqa: E402
from concourse import mybir  # noqa: E402
from concourse.bass_utils import run_bass_kernel_spmd  # noqa: E402

f32 = mybir.dt.float32
f32r = mybir.dt.float32r
bf16 = mybir.dt.bfloat16
fp8 = mybir.dt.float8e4
AF = mybir.ActivationFunctionType
ALU = mybir.AluOpType
PM = mybir.MatmulPerfMode
E4 = ml_dtypes.float8_e4m3

B, N, C = 8, 1024, 1024
H, D = 16, 64
HID = 4 * C
CT = C // 128           # 8 c-tiles
CP = CT // 2            # 4 c-tile pairs
NT = N // 128           # 8 n/k-tiles
HT = HID // 128         # 32 hid-tiles
HP = HT // 2            # 16 hid-tile pairs
SCALE = D ** -0.5
EPS = 1e-5
MASK_NEG = -10000.0
WS = 64.0               # fp8 weight scale
YS = 16.0               # attention-output scale into fp8

NCORES = 8


def _layer_norm(nc, tc, srcf, dstf, dst_dt, onesP, epsc, gc, bc, tag):
    """dstf(ct) <- LN(src) per token (free dim); srcf(ct) returns the ct'th
    [128, N] f32r slice (C on partitions). Stats via ones-matmul partition
    reductions broadcast to all 128 partitions."""
    ln_cm = tc.tile_pool(name=f"ln_{tag}", bufs=1)
    lnps_cm = tc.tile_pool(name=f"lnps_{tag}", bufs=1, space="PSUM")
    work = ln_cm.__enter__()
    ps = lnps_cm.__enter__()

    ps_sum = ps.tile([128, N], f32, tag="lnsum", name=f"ps_sum_{tag}")
    ps_sq = ps.tile([128, N], f32, tag="lnsq", name=f"ps_sq_{tag}")
    for ct in range(CT):
        sq = work.tile([128, N], f32r, tag="lnsqt", bufs=2, name=f"sq{tag}{ct}")
        nc.scalar.activation(out=sq, in_=srcf(ct), func=AF.Square)
        for ch in range(2):
            cs = slice(ch * 512, (ch + 1) * 512)
            nc.tensor.matmul(ps_sum[:, cs], lhsT=onesP, rhs=srcf(ct)[:, cs],
                             start=(ct == 0), stop=(ct == CT - 1))
            nc.tensor.matmul(ps_sq[:, cs], lhsT=onesP, rhs=sq[:, cs],
                             start=(ct == 0), stop=(ct == CT - 1))
    meanB = work.tile([128, N], f32, tag="meanB", name=f"meanB_{tag}")
    nc.vector.tensor_scalar_mul(meanB, ps_sum, 1.0 / C)
    msq = work.tile([128, N], f32, tag="msq", name=f"msq_{tag}")
    nc.scalar.activation(out=msq, in_=ps_sum, func=AF.Square, scale=1.0 / C)
    varB = work.tile([128, N], f32, tag="varB", name=f"varB_{tag}")
    nc.vector.scalar_tensor_tensor(varB, in0=ps_sq, scalar=1.0 / C, in1=msq,
                                   op0=ALU.mult, op1=ALU.subtract)
    stdB = work.tile([128, N], f32, tag="msq", name=f"stdB_{tag}")
    nc.scalar.activation(out=stdB, in_=varB, func=AF.Sqrt, bias=epsc,
                         scale=1.0)
    rstdB = work.tile([128, N], f32, tag="rstdB", name=f"rstdB_{tag}")
    nc.vector.reciprocal(rstdB, stdB)
    for ct in range(CT):
        d = work.tile([128, N], f32, tag="lnd", bufs=2, name=f"lnd{tag}{ct}")
        nc.vector.tensor_sub(d, srcf(ct), meanB)
        t = work.tile([128, N], f32, tag="lnt", bufs=2, name=f"lnt{tag}{ct}")
        nc.vector.scalar_tensor_tensor(t, in0=d, scalar=gc[:, ct:ct + 1],
                                       in1=rstdB, op0=ALU.mult, op1=ALU.mult)
        nc.scalar.activation(out=dstf(ct), in_=t, func=AF.Identity,
                             bias=bc[:, ct:ct + 1], scale=1.0)

    lnps_cm.__exit__(None, None, None)
    ln_cm.__exit__(None, None, None)


def _ln_from_stats(nc, work, ps_sum, ps_sq, srcf, dstf, onesP, epsc, gc, bc,
                   tag):
    meanB = work.tile([128, N], f32, tag="meanB", name=f"meanB_{tag}")
    nc.vector.tensor_scalar_mul(meanB, ps_sum, 1.0 / C)
    msq = work.tile([128, N], f32, tag="msq", name=f"msq_{tag}")
    nc.scalar.activation(out=msq, in_=ps_sum, func=AF.Square, scale=1.0 / C)
    varB = work.tile([128, N], f32, tag="varB", name=f"varB_{tag}")
    nc.vector.scalar_tensor_tensor(varB, in0=ps_sq, scalar=1.0 / C, in1=msq,
                                   op0=ALU.mult, op1=ALU.subtract)
    stdB = work.tile([128, N], f32, tag="msq", name=f"stdB_{tag}")
    nc.scalar.activation(out=stdB, in_=varB, func=AF.Sqrt, bias=epsc,
                         scale=1.0)
    rstdB = work.tile([128, N], f32, tag="rstdB", name=f"rstdB_{tag}")
    nc.vector.reciprocal(rstdB, stdB)
    for ct in range(CT):
        d = work.tile([128, N], f32, tag="lnd", bufs=2, name=f"lnd{tag}{ct}")
        nc.vector.tensor_sub(d, srcf(ct), meanB)
        t = work.tile([128, N], f32, tag="lnt", bufs=2, name=f"lnt{tag}{ct}")
        nc.vector.scalar_tensor_tensor(t, in0=d, scalar=gc[:, ct:ct + 1],
                                       in1=rstdB, op0=ALU.mult, op1=ALU.mult)
        nc.scalar.activation(out=dstf(ct), in_=t, func=AF.Identity,
                             bias=bc[:, ct:ct + 1], scale=1.0)


def build_program(repeat=1):
    nc = bacc.Bacc("TRN2", target_bir_lowering=False, debug=False)

    xT = nc.dram_tensor("xT", [C, N], f32, kind="ExternalInput").ap()
    maskv = nc.dram_tensor("maskv", [N], f32, kind="ExternalInput").ap()
    g1 = nc.dram_tensor("g1", [C], f32, kind="ExternalInput").ap()
    b1 = nc.dram_tensor("b1", [C], f32, kind="ExternalInput").ap()
    g2 = nc.dram_tensor("g2", [C], f32, kind="ExternalInput").ap()
    b2 = nc.dram_tensor("b2", [C], f32, kind="ExternalInput").ap()
    bproj = nc.dram_tensor("bproj", [C], f32, kind="ExternalInput").ap()
    bb1 = nc.dram_tensor("bb1", [HID], f32, kind="ExternalInput").ap()
    bb2 = nc.dram_tensor("bb2", [C], f32, kind="ExternalInput").ap()
    # fp8 weights, SBUF-layout ([128 partitions, ...]) packed host-side
    wqk8 = nc.dram_tensor("wqk8", [128, 16, CP, 256], fp8,
                          kind="ExternalInput").ap()
    wv8 = nc.dram_tensor("wv8", [128, CP, 2, 1024], fp8,
                         kind="ExternalInput").ap()
    wp8 = nc.dram_tensor("wp8", [128, CT, CP, 256], fp8,
                         kind="ExternalInput").ap()
    w2hi = nc.dram_tensor("w2hi", [128, CT, HP, 256], fp8,
                          kind="ExternalInput").ap()
    w2lo = nc.dram_tensor("w2lo", [128, CT, HP, 256], fp8,
                          kind="ExternalInput").ap()
    w1b = nc.dram_tensor("w1b", [8, CT, 128, 512], bf16,
                         kind="ExternalInput").ap()
    onesd = nc.dram_tensor("onesd", [1, 128], f32, kind="ExternalInput").ap()
    outT = nc.dram_tensor("outT", [C, N], f32, kind="ExternalOutput").ap()
    rrd = nc.dram_tensor("rrd", [H, N], f32)  # denom-reciprocal bounce

    # [C, N] DRAM tensors viewed as two [128, 4, N] row-groups for merged DMA
    def rg(ap_, half):
        return ap_[half * 512:(half + 1) * 512, :].rearrange(
            "(a p) f -> p a f", p=128)

    with tile.TileContext(nc) as tc:
        const_cm = tc.tile_pool(name="const", bufs=1)
        const = const_cm.__enter__()

        def vec_tiles(src_ap, n_t, name):
            t = const.tile([128, n_t], f32, name=name)
            nc.sync.dma_start(out=t, in_=src_ap.rearrange("(t p) -> p t", p=128))
            return t

        g1c = vec_tiles(g1, CT, "g1c")
        b1c = vec_tiles(b1, CT, "b1c")
        g2c = vec_tiles(g2, CT, "g2c")
        b2c = vec_tiles(b2, CT, "b2c")
        bprojc = vec_tiles(bproj, CT, "bprojc")
        bb1c = vec_tiles(bb1, HT, "bb1c")
        bb2c = vec_tiles(bb2, CT, "bb2c")
        maskc = vec_tiles(maskv, NT, "maskc")
        onesP = const.tile([128, 128], f32r, name="onesP")
        nc.sync.dma_start(out=onesP,
                          in_=onesd.partition_broadcast(128).bitcast(f32r))
        epsc = const.tile([128, 1], f32, name="epsc")
        nc.vector.memset(epsc, EPS)

        hoist = os.environ.get("KHOIST") == "1" and repeat > 1
        if hoist:
            hw_cm = tc.tile_pool(name="hoist_w", bufs=1)
            hwp = hw_cm.__enter__()
            h_wqk = hwp.tile([128, 16, CP, 256], fp8, name="h_wqk")
            nc.sync.dma_start(out=h_wqk[:, 0:8], in_=wqk8[:, 0:8])
            nc.sync.dma_start(out=h_wqk[:, 8:16], in_=wqk8[:, 8:16])
            h_wv = hwp.tile([128, CP, 2, 1024], fp8, name="h_wv")
            nc.gpsimd.dma_start(out=h_wv, in_=wv8)
            h_wp = hwp.tile([128, CT, CP, 256], fp8, name="h_wp")
            nc.sync.dma_start(out=h_wp, in_=wp8)
            h_w2hi = hwp.tile([128, CT, HP, 256], fp8, name="h_w2hi")
            h_w2lo = hwp.tile([128, CT, HP, 256], fp8, name="h_w2lo")
            for hh in range(2):
                nc.scalar.dma_start(out=h_w2hi[:, 4 * hh:4 * hh + 4],
                                    in_=w2hi[:, 4 * hh:4 * hh + 4])
                nc.sync.dma_start(out=h_w2lo[:, 4 * hh:4 * hh + 4],
                                  in_=w2lo[:, 4 * hh:4 * hh + 4])
            h_w1g = []
            for fg in range(8):
                tls = []
                for cg in range(2):
                    wt = hwp.tile([128, 4, 512], bf16, name=f"hw1_{fg}_{cg}")
                    nc.gpsimd.dma_start(
                        out=wt,
                        in_=w1b[fg, 4 * cg:4 * cg + 4].rearrange(
                            "a p f -> p a f"))
                    tls.append(wt)
                h_w1g.append(tls)

        for _rep in range(repeat):
            # ============ LN1 (x -> xn fp8 pairs) ====================
            # right-side pools, pushed in reverse order of lifetime
            px2t_cm = tc.tile_pool(name="p_x2t", bufs=1, side="right")
            px2t = px2t_cm.__enter__()
            x2tB = [px2t.tile([128, 4, N], f32r, tag="x2t", bufs=2,
                              name=f"x2t{g}") for g in range(2)]

            def x2slice(ct):
                return x2tB[ct // 4][:, ct % 4, :]

            pxts_cm = tc.tile_pool(name="p_xts", bufs=1, side="right")
            pxts = pxts_cm.__enter__()
            xtsB = []
            for g in range(2):
                t = pxts.tile([128, 4, N], f32r, tag="xts", bufs=2,
                              name=f"xts{g}")
                nc.scalar.dma_start(out=t, in_=rg(xT, g).bitcast(f32r))
                xtsB.append(t)

            def xslice(ct):
                return xtsB[ct // 4][:, ct % 4, :]

            pln1_cm = tc.tile_pool(name="p_ln1", bufs=1, side="left")
            pln1 = pln1_cm.__enter__()
            xn2 = [pln1.tile([128, 2, N], fp8, tag="xn2", bufs=CP,
                             name=f"xn2_{cp}") for cp in range(CP)]

            _layer_norm(nc, tc, xslice,
                        lambda ct: xn2[ct // 2][:, ct % 2, :], fp8,
                        onesP, epsc, g1c, b1c, "ln1")

            # ======================== QKV ============================
            pyt_cm = tc.tile_pool(name="p_yt", bufs=1, side="right")
            pyt = pyt_cm.__enter__()
            yt2 = [pyt.tile([128, 2, N], fp8, tag="yt2", bufs=CP,
                            name=f"yt2_{cp}") for cp in range(CP)]
            if hoist:
                wp_t = h_wp
            else:
                wp_t = pyt.tile([128, CT, CP, 256], fp8, name="wp_t")
                nc.sync.dma_start(out=wp_t, in_=wp8)

            pqk_cm = tc.tile_pool(name="p_qk", bufs=1, side="right")
            pqk = pqk_cm.__enter__()
            qkt = [pqk.tile([128, N], bf16, tag="qkt", bufs=16,
                            name=f"qkt{i}") for i in range(16)]
            vk2 = [pqk.tile([128, 2, H * (D + 1)], fp8, tag="vk2", bufs=4,
                            name=f"vk2_{kp}") for kp in range(CP)]
            for kp in range(CP):
                vcol = vk2[kp].rearrange("p t (h u) -> p t h u", u=D + 1)
                nc.vector.memset(vcol[:, :, :, D:D + 1], 1.0)

            wq_cm = tc.tile_pool(name="wq_pool", bufs=1)
            wqp = wq_cm.__enter__()
            if hoist:
                wqk_t, wv_t = h_wqk, h_wv
            else:
                wqk_t = wqp.tile([128, 16, CP, 256], fp8, name="wqk_t")
                nc.sync.dma_start(out=wqk_t[:, 0:8], in_=wqk8[:, 0:8])
                nc.sync.dma_start(out=wqk_t[:, 8:16], in_=wqk8[:, 8:16])
                wv_t = wqp.tile([128, CP, 2, 1024], fp8, name="wv_t")
                nc.gpsimd.dma_start(out=wv_t, in_=wv8)

            qps_cm = tc.tile_pool(name="qkv_ps", bufs=1, space="PSUM")
            qps = qps_cm.__enter__()

            def wqk_l(ot, cp):
                return wqk_t[:, ot, cp, :].rearrange("p (t f) -> p t f", t=2)

            def qk_one(ot):
                ps = qps.tile([128, N], f32, tag="qkvps", bufs=3,
                              name=f"qkps{ot}")
                for cp in range(CP):
                    for ch in range(2):
                        cs = slice(ch * 512, (ch + 1) * 512)
                        nc.tensor.matmul(ps[:, cs], lhsT=wqk_l(ot, cp),
                                         rhs=xn2[cp][:, :, cs],
                                         start=(cp == 0), stop=(cp == CP - 1),
                                         perf_mode=PM.DoubleRow)
                nc.vector.tensor_copy(qkt[ot], ps)

            def v_one(nt):
                ps = qps.tile([128, N], f32, tag="qkvps", bufs=3,
                              name=f"vps{nt}")
                ns = slice(nt * 128, (nt + 1) * 128)
                for cp in range(CP):
                    for ch in range(2):
                        cs = slice(ch * 512, (ch + 1) * 512)
                        nc.tensor.matmul(
                            ps[:, cs], lhsT=xn2[cp][:, :, ns],
                            rhs=wv_t[:, cp, ch, :].rearrange(
                                "p (t f) -> p t f", t=2),
                            start=(cp == 0), stop=(cp == CP - 1),
                            perf_mode=PM.DoubleRow)
                dst = vk2[nt // 2].rearrange(
                    "p t (h u) -> p t h u", u=D + 1)[:, nt % 2, :, 0:D]
                nc.scalar.activation(
                    out=dst, in_=ps.rearrange("p (h d) -> p h d", d=D),
                    func=AF.Identity, scale=1.0 / WS)

            # q0,k0 first so attention can start; v next (AV h0 needs all v)
            qk_one(0)
            qk_one(8)
            for nt in range(NT):
                v_one(nt)
            for j in range(1, 8):
                qk_one(j)
                qk_one(8 + j)

            qps_cm.__exit__(None, None, None)
            wq_cm.__exit__(None, None, None)
            pln1_cm.__exit__(None, None, None)

            # ====================== attention ========================
            asb_cm = tc.tile_pool(name="attn_sb", bufs=1)
            asb = asb_cm.__enter__()
            aps_cm = tc.tile_pool(name="attn_ps", bufs=1, space="PSUM")
            aps = aps_cm.__enter__()

            def qk_exp(h):
                """QK^T matmuls + exp for head h; returns ea pair tiles."""
                j, half = h // 2, h % 2
                hs = slice(half * D, (half + 1) * D)
                qk_q = qkt[j]
                qk_k = qkt[8 + j]
                ea2 = []
                for kt in range(NT):
                    ks = slice(kt * 128, (kt + 1) * 128)
                    sa = aps.tile([128, N], f32, tag="sa", bufs=3,
                                  name=f"sa{h}_{kt}")
                    for ch in range(2):
                        cs = slice(ch * 512, (ch + 1) * 512)
                        nc.tensor.matmul(sa[:, cs], lhsT=qk_k[hs, ks],
                                         rhs=qk_q[hs, cs],
                                         start=True, stop=True)
                    if kt % 2 == 0:
                        ea = asb.tile([128, 2, N], fp8, tag="ea", bufs=10,
                                      name=f"ea{h}_{kt // 2}")
                        ea2.append(ea)
                    nc.scalar.activation(out=ea2[kt // 2][:, kt % 2, :],
                                         in_=sa, func=AF.Exp,
                                         bias=maskc[:, kt:kt + 1],
                                         scale=SCALE / (WS * WS))
                return ea2

            def av_norm(h, ea2):
                """AV + softmax-normalize for head h."""
                yach = [aps.tile([D + 1, 512], f32, tag=f"ya{ch}", bufs=1,
                                 name=f"ya{h}_{ch}") for ch in range(2)]
                for kp in range(CP):
                    va = vk2[kp][:, :, h * 65:h * 65 + 65]
                    for ch in range(2):
                        cs = slice(ch * 512, (ch + 1) * 512)
                        nc.tensor.matmul(yach[ch], lhsT=va,
                                         rhs=ea2[kp][:, :, cs],
                                         start=(kp == 0), stop=(kp == CP - 1),
                                         perf_mode=PM.DoubleRow)
                yu = asb.tile([D + 1, N], f32, tag="yu", bufs=4,
                              name=f"yu{h}")
                for ch in range(2):
                    nc.vector.tensor_copy(yu[:, ch * 512:(ch + 1) * 512],
                                          yach[ch])
                rr = asb.tile([1, N], f32, tag="rr", bufs=6, name=f"rr{h}")
                nc.vector.reciprocal(rr, yu[D:D + 1, :])
                row = rrd.ap()[h:h + 1, :]
                nc.sync.dma_start(out=row, in_=rr)
                rb = asb.tile([D, N], f32, tag="rb", bufs=4, name=f"rb{h}")
                nc.sync.dma_start(out=rb, in_=row.partition_broadcast(D))
                # yt = y*YS/denom, fp8
                nc.vector.scalar_tensor_tensor(
                    yt2[h // 4][(h % 2) * D:(h % 2) * D + D, (h // 2) % 2, :],
                    in0=yu[0:D, :], scalar=YS, in1=rb,
                    op0=ALU.mult, op1=ALU.mult)

            # two heads in flight: QK(h+1) issues while exps(h) drain, so
            # AV(h) never exposes the last-exp handoff latency on the PE
            ea_prev = qk_exp(0)
            for h in range(H):
                if h + 1 < H:
                    ea_next = qk_exp(h + 1)
                else:
                    ea_next = None
                av_norm(h, ea_prev)
                ea_prev = ea_next

            aps_cm.__exit__(None, None, None)
            asb_cm.__exit__(None, None, None)
            pqk_cm.__exit__(None, None, None)

            # ============ proj + residual; prefetch MLP weights ======
            mw_cm = tc.tile_pool(name="mw_pool", bufs=1)
            mwp = mw_cm.__enter__()
            if hoist:
                w1groups = h_w1g
                w2hi_t, w2lo_t = h_w2hi, h_w2lo
            else:
                # w1 on gpsimd; w2hi on scalar, w2lo on sync
                w1groups = []
                for fg in range(8):
                    w1tiles = []
                    for cg in range(2):
                        wt = mwp.tile([128, 4, 512], bf16, tag="w1",
                                      bufs=4, name=f"w1_{fg}_{cg}")
                        nc.gpsimd.dma_start(
                            out=wt,
                            in_=w1b[fg, 4 * cg:4 * cg + 4].rearrange(
                                "a p f -> p a f"))
                        w1tiles.append(wt)
                    w1groups.append(w1tiles)
                w2hi_t = mwp.tile([128, CT, HP, 256], fp8, name="w2hi_t")
                w2lo_t = mwp.tile([128, CT, HP, 256], fp8, name="w2lo_t")
                for hh in range(2):
                    nc.scalar.dma_start(out=w2hi_t[:, 4 * hh:4 * hh + 4],
                                        in_=w2hi[:, 4 * hh:4 * hh + 4])
                    nc.sync.dma_start(out=w2lo_t[:, 4 * hh:4 * hh + 4],
                                      in_=w2lo[:, 4 * hh:4 * hh + 4])

            pps_cm = tc.tile_pool(name="proj_ps", bufs=1, space="PSUM")
            pps = pps_cm.__enter__()
            # LN2 stats psums + sq tiles interleaved with the proj loop so
            # the reduction starts as each x2 tile lands
            ln2w_cm = tc.tile_pool(name="ln2_w", bufs=1)
            ln2w = ln2w_cm.__enter__()
            ps2_sum = pps.tile([128, N], f32, tag="ln2sum", name="ps2_sum")
            ps2_sq = pps.tile([128, N], f32, tag="ln2sq", name="ps2_sq")
            for o in range(CT):
                ps = pps.tile([128, N], f32, tag="projps", bufs=2,
                              name=f"pps{o}")
                for cp in range(CP):
                    lw = wp_t[:, o, cp, :].rearrange("p (t f) -> p t f", t=2)
                    for ch in range(2):
                        cs = slice(ch * 512, (ch + 1) * 512)
                        nc.tensor.matmul(
                            ps[:, cs], lhsT=lw, rhs=yt2[cp][:, :, cs],
                            start=(cp == 0), stop=(cp == CP - 1),
                            perf_mode=PM.DoubleRow)
                # x2 = ps/(WS*YS) + x, then += bproj
                xs = x2slice(o)
                nc.vector.scalar_tensor_tensor(
                    xs, in0=ps, scalar=1.0 / (WS * YS),
                    in1=xslice(o), op0=ALU.mult, op1=ALU.add)
                nc.vector.tensor_scalar_add(xs, xs, bprojc[:, o:o + 1])
                sq = ln2w.tile([128, N], f32r, tag="ln2sqt", bufs=2,
                               name=f"sq2_{o}")
                nc.scalar.activation(out=sq, in_=xs, func=AF.Square)
                for ch in range(2):
                    cs = slice(ch * 512, (ch + 1) * 512)
                    nc.tensor.matmul(ps2_sum[:, cs], lhsT=onesP,
                                     rhs=xs[:, cs],
                                     start=(o == 0), stop=(o == CT - 1))
                    nc.tensor.matmul(ps2_sq[:, cs], lhsT=onesP,
                                     rhs=sq[:, cs],
                                     start=(o == 0), stop=(o == CT - 1))
            pyt_cm.__exit__(None, None, None)

            # =================== LN2 (x2 -> x2n bf16) ================
            px2n_cm = tc.tile_pool(name="p_x2n", bufs=1, side="left")
            px2n = px2n_cm.__enter__()
            x2nb = [px2n.tile([128, N], bf16, tag="x2n", bufs=CT,
                              name=f"x2n{ct}") for ct in range(CT)]
            _ln_from_stats(nc, ln2w, ps2_sum, ps2_sq, x2slice,
                           lambda ct: x2nb[ct], onesP, epsc, g2c, b2c, "ln2")
            pps_cm.__exit__(None, None, None)
            pxts_cm.__exit__(None, None, None)

            # ======== MLP: fc1 bf16, fc2 fp8 weight-compensated ======
            pmlp_cm = tc.tile_pool(name="p_mlp", bufs=1, side="right")
            pmlp = pmlp_cm.__enter__()
            h2 = [pmlp.tile([128, 2, N], fp8, tag="h2", bufs=HP,
                            name=f"h2_{fp}") for fp in range(HP)]
            mps_cm = tc.tile_pool(name="mlp_ps", bufs=1, space="PSUM")
            mps = mps_cm.__enter__()

            for f in range(HT):
                fg, fi = f // 4, f % 4
                w1tiles = w1groups[fg]
                ps = mps.tile([128, N], f32, tag="mlp1ps", bufs=2,
                              name=f"m1ps{f}")
                fs = slice(fi * 128, (fi + 1) * 128)
                for ct in range(CT):
                    for ch in range(2):
                        cs = slice(ch * 512, (ch + 1) * 512)
                        nc.tensor.matmul(
                            ps[:, cs], lhsT=w1tiles[ct // 4][:, ct % 4, fs],
                            rhs=x2nb[ct][:, cs],
                            start=(ct == 0), stop=(ct == CT - 1))
                nc.scalar.activation(out=h2[f // 2][:, f % 2, :], in_=ps,
                                     func=AF.Gelu, bias=bb1c[:, f:f + 1],
                                     scale=1.0)
            px2n_cm.__exit__(None, None, None)
            ln2w_cm.__exit__(None, None, None)

            # fc2 + bias + residual out
            osb_cm = tc.tile_pool(name="out_sb", bufs=1)
            osb = osb_cm.__enter__()
            ot = [osb.tile([128, 4, N], f32, tag="ot", bufs=2, name=f"ot{g}")
                  for g in range(2)]

            def w2_l(wt, o, hp):
                return wt[:, o, hp, :].rearrange("p (t f) -> p t f", t=2)

            for o in range(CT):
                pm = mps.tile([128, N], f32, tag="pm", bufs=2, name=f"pm{o}")
                for hp in range(HP):
                    for wt in (w2hi_t, w2lo_t):
                        lw = w2_l(wt, o, hp)
                        for ch in range(2):
                            cs = slice(ch * 512, (ch + 1) * 512)
                            nc.tensor.matmul(
                                pm[:, cs], lhsT=lw, rhs=h2[hp][:, :, cs],
                                start=(hp == 0 and wt is w2hi_t),
                                stop=(hp == HP - 1 and wt is w2lo_t),
                                perf_mode=PM.DoubleRow)
                dst = ot[o // 4][:, o % 4, :]
                nc.vector.scalar_tensor_tensor(
                    dst, in0=pm, scalar=1.0 / WS,
                    in1=x2slice(o), op0=ALU.mult, op1=ALU.add)
                nc.vector.tensor_scalar_add(dst, dst, bb2c[:, o:o + 1])
            for g in range(2):
                nc.scalar.dma_start(out=rg(outT, g), in_=ot[g])

            mps_cm.__exit__(None, None, None)
            osb_cm.__exit__(None, None, None)
            mw_cm.__exit__(None, None, None)
            pmlp_cm.__exit__(None, None, None)
            px2t_cm.__exit__(None, None, None)

        if hoist:
            hw_cm.__exit__(None, None, None)
        const_cm.__exit__(None, None, None)

    nc.compile()
    return nc


_NC_CACHE = {}


def _get_program():
    if "nc" not in _NC_CACHE:
        _NC_CACHE["nc"] = build_program()
    return _NC_CACHE["nc"]


def _prep_weights(Wqkv, Wproj, W1, W2):
    Wqkv = np.asarray(Wqkv, np.float32)
    Wproj = np.asarray(Wproj, np.float32)
    W1 = np.asarray(W1, np.float32)
    W2 = np.asarray(W2, np.float32)

    # q/k stationary: [p, ot, cp, t*f]; val = Wqk[ot*128+f, cp*256+t*128+p]*WS
    a = (Wqkv[:2 * C] * WS).reshape(16, 128, CP, 2, 128)   # [ot, f, cp, t, p]
    wqk8 = np.ascontiguousarray(
        a.transpose(4, 0, 2, 3, 1)).astype(E4).reshape(128, 16, CP, 256)
    # v moving: [p, cp, ch, t*n]; val = Wv[ch*512+n, cp*256+t*128+p]*WS
    a = (Wqkv[2 * C:] * WS).reshape(2, 512, CP, 2, 128)    # [ch, n, cp, t, p]
    wv8 = np.ascontiguousarray(
        a.transpose(4, 2, 0, 3, 1)).astype(E4).reshape(128, CP, 2, 1024)
    # proj stationary: [p, o, cp, t*f]
    a = (Wproj * WS).reshape(CT, 128, CP, 2, 128)          # [o, f, cp, t, p]
    wp8 = np.ascontiguousarray(
        a.transpose(4, 0, 2, 3, 1)).astype(E4).reshape(128, CT, CP, 256)
    # fc1 bf16, baseline-style [8, CT, 128, 512]
    W1T = np.ascontiguousarray(W1.T)                       # [C, HID]
    w1b = np.ascontiguousarray(
        W1T.reshape(CT, 128, 8, 512).transpose(2, 0, 1, 3)).astype(
        ml_dtypes.bfloat16)
    # fc2 stationary hi/lo: [p, o, hp, t*f]; val = W2[o*128+f, hp*256+t*128+p]
    a = (W2 * WS).reshape(CT, 128, HP, 2, 128)             # [o, f, hp, t, p]
    a = np.ascontiguousarray(a.transpose(4, 0, 2, 3, 1))   # [p, o, hp, t, f]
    hi = a.astype(E4)
    lo = (a - hi.astype(np.float32)).astype(E4)
    return {
        "wqk8": wqk8, "wv8": wv8, "wp8": wp8, "w1b": w1b,
        "w2hi": hi.reshape(128, CT, HP, 256),
        "w2lo": lo.reshape(128, CT, HP, 256),
    }


def kernel(x, length, g1, b1, Wqkv, Wproj, bproj, g2, b2, W1, bb1, W2, bb2):
    x = np.asarray(x, dtype=np.float32)
    length = np.asarray(length)
    g1 = np.asarray(g1, np.float32); b1 = np.asarray(b1, np.float32)
    g2 = np.asarray(g2, np.float32); b2 = np.asarray(b2, np.float32)
    bproj = np.asarray(bproj, np.float32)
    bb1 = np.asarray(bb1, np.float32); bb2 = np.asarray(bb2, np.float32)

    wts = _prep_weights(Wqkv, Wproj, W1, W2)
    xT = np.ascontiguousarray(x.transpose(0, 2, 1))  # [B, C, N]
    mask = (np.arange(N)[None, :] >= np.asarray(length)[:, None]).astype(
        np.float32) * MASK_NEG  # [B, N]

    shared = {"g1": g1, "b1": b1, "g2": g2, "b2": b2, "bproj": bproj,
              "bb1": bb1, "bb2": bb2, "onesd": np.ones((1, 128), np.float32),
              **wts}
    in_maps = [dict(shared, xT=xT[b], maskv=np.ascontiguousarray(mask[b]))
               for b in range(B)]

    nc = _get_program()
    res = run_bass_kernel_spmd(nc, in_maps, core_ids=list(range(NCORES)))
    out = np.stack([res.results[b]["outT"] for b in range(B)], axis=0)
    return np.ascontiguousarray(out.transpose(0, 2, 1))
